# revision 29
# baseline (speedup 1.0000x reference)
"""Distributed GAT (2-layer, BN between) on 8 Trainium2 NeuronCores.

Strategy (v2):
- Phase A replicated: every core computes the FULL z1/as1 gather table
  (x @ [W1|Wa1s] over all 50k nodes, streamed in 8 xT chunks) into local
  DRAM -> no tb1 AllGather. ad1 for the core's own dst shard comes from a
  separate per-core xTme input (SPMD-safe).
- Edges sorted by dst, partitioned to the owning core, grouped in 128-edge
  blocks per 128-dst tile, split in 2 sections by src < 25000 (int16 gather
  range). ONE dma_gather call per (tile, section) (fixed SWDGE call
  overhead ~1us dominates); padding slots use idx=-1 (trailing skip, no
  descriptors/bytes), with valid counts equalized across cores via idx=0
  dummies so num_idxs_reg is a compile-time constant.
- Attention p = exp(leaky(as[src]+ad[dst])) built on-chip; ad[dst] per edge
  via selection-mask matmul (maskT); scatter-add via maskE matmuls
  accumulating in PSUM. Masks are fp8 (e4m3) to halve their DMA stream.
- p overwrites the as-slot in the gathered rows so the scatter matmul
  carries [messages | p] in one pass; denominators land in the same PSUM.
- BatchNorm statistics via ones-matmul + AllReduce; affine folded.
- y/z2 in transposed layout (DMA-transpose); leaky via scalar Lrelu.
- L2 table z2 = y @ [W2|Wa2s|Wa2d] per shard + AllGather (12.8MB); L2 edge
  pass mirrors L1 with 1 head and 256B rows.
"""
import sys
import types

sys.path.insert(0, "/opt/trn_rl_repo")

import numpy as np
import ml_dtypes

# antenv.axon_hooks shim (needed only when tracing; harmless otherwise)
try:
    import antenv.axon_hooks  # noqa: F401
except Exception:
    try:
        import antenv

        _m = types.ModuleType("antenv.axon_hooks")
        _m._hook = None

        def _set(h):
            _m._hook = h

        def _get():
            return _m._hook

        _m.set_axon_ntff_profile_hook = _set
        _m.get_axon_ntff_profile_hook = _get
        sys.modules["antenv.axon_hooks"] = _m
        antenv.axon_hooks = _m
    except Exception:
        pass

import concourse.bacc as bacc
import concourse.mybir as mybir
import concourse.tile as tile
from concourse import bass_utils

F32 = mybir.dt.float32
F16 = mybir.dt.float16
F8 = mybir.dt.float8e4
I16 = mybir.dt.int16
OP = mybir.AluOpType
ACTF = mybir.ActivationFunctionType

N, E, F_IN, HID, HEADS, CLASSES = 50000, 800000, 128, 64, 4, 64
R = 8                      # cores
NS = N // R                # nodes per shard (6250)
NT = (NS + 127) // 128     # dst tiles per shard (49)
SECT = 25000               # gather-table section split (int16 index range)
HC = HEADS * HID           # 256
ROW1 = 384                 # halves per L1 table row: z(256) | as f32(8) | pad
ROW2 = 128                 # halves per L2 table row: z2(64) | as2 f32(2) | pad
W2C = CLASSES + 2          # 66
NEG_ATT = 0.2
NEG_ACT = 0.01
BN_EPS = 1e-5
MAXK = 24                  # max blocks per tile (both sections)
MAXBLK = 8                 # max blocks per dma_gather call (ring capacity)
NQ = 4                     # SWDGE queues
F8M = True                # fp8 masks (else f16)
SIM_SAFE = False           # full-width table writes (CoreSim uninit checks)
LRELU = False              # scalar-engine Lrelu/Square (else DVE ops)


def _tile_nodes(t):
    return 128 if t < NT - 1 else NS - 128 * (NT - 1)


def plan(edge_index):
    """Host-side edge partitioning. Returns the (core-independent) schedule and
    per-core packed arrays."""
    ei = np.asarray(edge_index)
    src = np.concatenate([ei[0], np.arange(N, dtype=np.int64)]).astype(np.int64)
    dst = np.concatenate([ei[1], np.arange(N, dtype=np.int64)]).astype(np.int64)
    order = np.argsort(dst, kind="stable")
    src, dst = src[order], dst[order]

    # split each (core, tile) range, then sections by src < SECT
    core_of = dst // NS
    core_bounds = np.searchsorted(core_of, np.arange(R + 1))
    per = []  # per core: list over tiles of (srcA, dstA, srcB, dstB)
    for c in range(R):
        s0, s1 = core_bounds[c], core_bounds[c + 1]
        sc, dc = src[s0:s1], dst[s0:s1] - c * NS
        tb = np.searchsorted(dc // 128, np.arange(NT + 1))
        tiles = []
        for t in range(NT):
            st, dt_ = sc[tb[t]:tb[t + 1]], dc[tb[t]:tb[t + 1]] - t * 128
            a = st < SECT
            tiles.append((st[a], dt_[a], st[~a] - SECT, dt_[~a]))
        per.append(tiles)

    # common schedule: per (tile, sec): max valid count V over cores and
    # block count k = ceil(V/128)
    sched = []
    blk0 = 0
    for t in range(NT):
        vA = max(len(per[c][t][0]) for c in range(R))
        vB = max(len(per[c][t][2]) for c in range(R))
        kA = -(-vA // 128)
        kB = -(-vB // 128)
        assert kA + kB <= MAXK
        sched.append({"t": t, "blk0": blk0, "nb": kA + kB,
                      "kA": kA, "kB": kB, "vA": vA, "vB": vB})
        blk0 += kA + kB
    nblk = blk0

    # pack per-core arrays
    packs = []
    for c in range(R):
        idx = np.full((nblk * 128,), -1, dtype=np.int16)
        dloc = np.full((nblk * 128,), -1.0, dtype=np.float32)
        for S in sched:
            t, b0, kA = S["t"], S["blk0"], S["kA"]
            sA, dA, sB, dB = per[c][t]
            for (ss, dd), koff, V in (((sA, dA), 0, S["vA"]),
                                      ((sB, dB), kA, S["vB"])):
                o = (b0 + koff) * 128
                idx[o:o + len(ss)] = ss.astype(np.int16)
                idx[o + len(ss):o + V] = 0          # dummy-valid pad (masked)
                dloc[o:o + len(ss)] = dd.astype(np.float32)
        # maskT [128 dloc, nblk, 128 slot]; maskE [128 slot, nblk, 128 dloc] fp8
        mdt = ml_dtypes.float8_e4m3 if F8M else np.float16
        maskT = np.zeros((128, nblk, 128), dtype=mdt)
        maskE = np.zeros((128, nblk, 128), dtype=mdt)
        j = np.nonzero(dloc >= 0)[0]
        maskT[dloc[j].astype(np.int64), j // 128, j % 128] = 1.0
        maskE[j % 128, j // 128, dloc[j].astype(np.int64)] = 1.0
        # idx wrapped: [16, ni/16] replicated to 128 partitions; 8 cols/block
        w = idx.reshape(nblk * 8, 16).T          # [16, nblk*8]
        idx128 = np.tile(w, (8, 1))
        packs.append({"idx": idx128, "maskT": maskT, "maskE": maskE})
    return sched, nblk, packs


def host_inputs(x, edge_index, W1, a_src1, a_dst1, gamma, beta, W2, a_src2, a_dst2, b2):
    sched, nblk, packs = plan(edge_index)
    x = np.asarray(x, dtype=np.float32)
    W1 = np.asarray(W1, dtype=np.float32)
    a_src1 = np.asarray(a_src1, dtype=np.float32)
    a_dst1 = np.asarray(a_dst1, dtype=np.float32)
    W2 = np.asarray(W2, dtype=np.float32)
    a_src2 = np.asarray(a_src2, dtype=np.float32)
    a_dst2 = np.asarray(a_dst2, dtype=np.float32)

    # Wa1s[f, h] = sum_c W1[f, h*HID + c] * a_src1[h, c]
    W1r = W1.reshape(F_IN, HEADS, HID)
    Wa1s = np.einsum("fhc,hc->fh", W1r, a_src1)
    Wa1d = np.einsum("fhc,hc->fh", W1r, a_dst1)
    W1e = np.concatenate([W1, Wa1s], axis=1).astype(np.float16)   # [128, 260]
    W1ad = Wa1d.astype(np.float16)                                # [128, 4]

    Wa2s = W2 @ a_src2[0]        # [256]
    Wa2d = W2 @ a_dst2[0]
    W2ext = np.concatenate([W2, Wa2s[:, None], Wa2d[:, None]], axis=1).astype(np.float16)  # [256, 66]

    ones16 = np.ones((128, 1), dtype=np.float16)
    gb_in = np.concatenate([np.asarray(gamma, np.float32), np.asarray(beta, np.float32)])[None, :]  # [1,512]
    b2rep = np.tile(np.asarray(b2, np.float32)[None, :], (128, 1))  # [128, 64]

    xT = np.ascontiguousarray(x.T).astype(np.float16)       # [128, 50000] shared
    ins = []
    for c in range(R):
        xTme = np.ascontiguousarray(xT[:, c * NS:(c + 1) * NS])  # [128, 6250]
        ins.append({
            "xT": xT,
            "xTme": xTme,
            "W1e": W1e,
            "W1ad": W1ad,
            "W2ext": W2ext,
            "ones16": ones16,
            "gb_in": gb_in,
            "b2rep": b2rep,
            "idx": packs[c]["idx"],
            "maskT": packs[c]["maskT"],
            "maskE": packs[c]["maskE"],
        })
    return sched, nblk, ins


NSP = NT * 128  # padded shard rows (6272)


def build_program(sched, nblk, qmap=None):
    nc = bacc.Bacc("TRN2", target_bir_lowering=False, debug=False,
                   num_devices=R, num_swdge_queues=NQ,
                   dynamic_dma_scratch_size=16384)
    xT_d = nc.dram_tensor("xT", [F_IN, N], F16, kind="ExternalInput")
    xTme_d = nc.dram_tensor("xTme", [F_IN, NS], F16, kind="ExternalInput")
    W1e_d = nc.dram_tensor("W1e", [F_IN, HC + HEADS], F16, kind="ExternalInput")
    W1ad_d = nc.dram_tensor("W1ad", [F_IN, HEADS], F16, kind="ExternalInput")
    W2e_d = nc.dram_tensor("W2ext", [HC, W2C], F16, kind="ExternalInput")
    ones16_d = nc.dram_tensor("ones16", [128, 1], F16, kind="ExternalInput")
    gbin_d = nc.dram_tensor("gb_in", [1, 2 * HC], F32, kind="ExternalInput")
    b2rep_d = nc.dram_tensor("b2rep", [128, CLASSES], F32, kind="ExternalInput")
    idx_d = nc.dram_tensor("idx", [128, nblk * 8], I16, kind="ExternalInput")
    FM = F8 if F8M else F16
    maskT_d = nc.dram_tensor("maskT", [128, nblk, 128], FM, kind="ExternalInput")
    maskE_d = nc.dram_tensor("maskE", [128, nblk, 128], FM, kind="ExternalInput")
    out_d = nc.dram_tensor("out", [NS, CLASSES], F32, kind="ExternalOutput")

    with tile.TileContext(nc) as tc:
        import contextlib
        ctx = contextlib.ExitStack()
        with ctx:
            cons = ctx.enter_context(tc.tile_pool(name="cons", bufs=1))
            dram = ctx.enter_context(tc.tile_pool(name="dram", bufs=1, space="DRAM"))
            xp = ctx.enter_context(tc.tile_pool(name="xp", bufs=2))
            rowp = ctx.enter_context(tc.tile_pool(name="rowp", bufs=4))
            gp = ctx.enter_context(tc.tile_pool(name="gp", bufs=2))
            mp = ctx.enter_context(tc.tile_pool(name="mp", bufs=4))
            ep = ctx.enter_context(tc.tile_pool(name="ep", bufs=2))
            gop = ctx.enter_context(tc.tile_pool(name="gop", bufs=2))
            yp = ctx.enter_context(tc.tile_pool(name="yp", bufs=2))
            ps = ctx.enter_context(tc.tile_pool(name="ps", bufs=2, space="PSUM"))
            ps1 = ctx.enter_context(tc.tile_pool(name="ps1", bufs=1, space="PSUM"))
            bnp = ctx.enter_context(tc.tile_pool(name="bnp", bufs=1, space="PSUM"))

            # ---- constants into SBUF
            xTme = cons.tile([F_IN, NS], F16)
            nc.sync.dma_start(out=xTme[:], in_=xTme_d[:, :])
            W1e = cons.tile([F_IN, HC + HEADS], F16)
            nc.sync.dma_start(out=W1e[:], in_=W1e_d[:, :])
            W1ad = cons.tile([F_IN, HEADS], F16)
            nc.sync.dma_start(out=W1ad[:], in_=W1ad_d[:, :])
            W2e0 = cons.tile([128, W2C], F16)
            nc.sync.dma_start(out=W2e0[:], in_=W2e_d[0:128, :])
            W2e1 = cons.tile([128, W2C], F16)
            nc.sync.dma_start(out=W2e1[:], in_=W2e_d[128:256, :])
            ones16 = cons.tile([128, 1], F16)
            nc.sync.dma_start(out=ones16[:], in_=ones16_d[:, :])
            gbrow = cons.tile([1, 2 * HC], F32)
            nc.sync.dma_start(out=gbrow[:], in_=gbin_d[:, :])
            b2rep = cons.tile([128, CLASSES], F32)
            nc.sync.dma_start(out=b2rep[:], in_=b2rep_d[:, :])
            idx_t = cons.tile([128, nblk * 8], I16)
            nc.sync.dma_start(out=idx_t[:], in_=idx_d[:, :])
            ad_sh = cons.tile([128, NT, HEADS], F32)
            ad2_sh = cons.tile([128, NT, 1], F32)
            nc.vector.memset(ad_sh[:], 0.0)
            nc.vector.memset(ad2_sh[:], 0.0)

            # ---- internal DRAM
            tb1A = dram.tile([SECT, ROW1], F16)
            tb1B = dram.tile([SECT, ROW1], F16)
            g_sh = dram.tile([NSP, HC], F16)
            ar_in = dram.tile([1, 2 * HC], F32)
            ar_out = dram.tile([1, 2 * HC], F32, addr_space="Shared")
            gb_d = dram.tile([1, 2 * HC], F32)
            tb2_sh = dram.tile([NS, ROW2], F16)
            tb2 = dram.tile([N, ROW2], F16, addr_space="Shared")

            # ================= Phase A (replicated): full z1/as1 table ======
            with nc.named_scope("phA"):
                for c in range(R):
                    xc = xp.tile([F_IN, NS], F16, tag="xc")
                    nc.sync.dma_start(out=xc[:], in_=xT_d[:, c * NS:(c + 1) * NS])
                    for t in range(NT):
                        nt = _tile_nodes(t)
                        z1 = ps.tile([128, HC + HEADS], F32, tag="zps")
                        nc.tensor.matmul(z1[0:nt, :], lhsT=xc[:, t * 128:t * 128 + nt],
                                         rhs=W1e[:], start=True, stop=True)
                        row = rowp.tile([128, ROW1], F16, tag="row1")
                        nc.scalar.copy(out=row[0:nt, 0:HC], in_=z1[0:nt, 0:HC])
                        nc.scalar.copy(
                            out=row[:].bitcast(F32)[0:nt, HC // 2:HC // 2 + HEADS],
                            in_=z1[0:nt, HC:HC + HEADS])
                        tbh = tb1A if c < R // 2 else tb1B
                        r0 = c * NS + t * 128 - (0 if c < R // 2 else SECT)
                        ww = ROW1 if SIM_SAFE else HC + 8
                        if SIM_SAFE:
                            nc.vector.memset(row[0:nt, HC + 8:ROW1], 0.0)
                        nc.sync.dma_start(out=tbh[r0:r0 + nt, 0:ww], in_=row[0:nt, 0:ww])
                # ad1 for own dst shard (per-core input, SPMD-safe)
                for t in range(NT):
                    nt = _tile_nodes(t)
                    adp = ps1.tile([128, MAXK * HEADS], F32, tag="adps")
                    nc.tensor.matmul(adp[0:nt, 0:HEADS], lhsT=xTme[:, t * 128:t * 128 + nt],
                                     rhs=W1ad[:], start=True, stop=True)
                    nc.vector.tensor_copy(out=ad_sh[0:nt, t, :], in_=adp[0:nt, 0:HEADS])

            gq = [0]  # build-order gather call counter
            gather_names = []

            def nextq():
                # queue per call from qmap (lane%4 of the scheduled order);
                # pass 1 (qmap None) uses queue 0 everywhere.
                q = qmap[gq[0]] if qmap is not None else 0
                gq[0] += 1
                return q

            def gather_calls(k, V):
                # split a (tile, sec) gather into calls of <= MAXBLK blocks;
                # each call: (block_off, nblocks, valid_count)
                calls = []
                off = 0
                while off < k:
                    nbc = min(MAXBLK, k - off)
                    vc = max(0, min(V - off * 128, nbc * 128))
                    calls.append((off, nbc, vc))
                    off += nbc
                return calls

            MAXH = max(max(S["kA"], S["kB"]) for S in sched)

            # ================= L1 edge pass (2 sub-passes) =================
            # Pass A processes every tile's section-A blocks (ready as soon
            # as tb1A is written, overlapping phase A's B-half) into an SBUF
            # f16 accumulator; pass B adds section-B and finalizes.
            def half_tile(S, sec, rows, mT, mE, out_ps, first):
                """Gather + attention + scatter for one (tile, section)."""
                t, b0 = S["t"], S["blk0"]
                if sec == 0:
                    k, V, boff, tbh = S["kA"], S["vA"], 0, tb1A
                else:
                    k, V, boff, tbh = S["kB"], S["vB"], S["kA"], tb1B
                if first:
                    nc.vector.memset(rows[:], 0.0)
                elif V < k * 128:
                    nc.vector.memset(rows[:, k - 1, :], 0.0)
                nc.sync.dma_start(out=mT[:, 0:k, :],
                                  in_=maskT_d[:, b0 + boff:b0 + boff + k, :])
                nc.sync.dma_start(out=mE[:, 0:k, :],
                                  in_=maskE_d[:, b0 + boff:b0 + boff + k, :])
                for (o, nbc, vc) in gather_calls(k, V):
                    gi = nc.gpsimd.dma_gather(
                        rows[:, o:o + nbc, :], tbh[0:SECT, :],
                        idx_t[:, (b0 + boff + o) * 8:(b0 + boff + o + nbc) * 8],
                        nbc * 128, vc, ROW1, queue_num=nextq())
                    gather_names.append(gi.ins.name)
                ad_ps = ps1.tile([128, MAXK * HEADS], F32, tag="adps")
                for b in range(k):
                    nc.tensor.matmul(ad_ps[:, b * HEADS:(b + 1) * HEADS],
                                     lhsT=mT[:, b, :], rhs=ad16_all[:, t, :],
                                     start=True, stop=True)
                e_t = ep.tile([128, MAXH, HEADS], F32, tag="e")
                nc.vector.tensor_tensor(
                    out=e_t[:, 0:k, :],
                    in0=rows[:].bitcast(F32)[:, 0:k, HC // 2:HC // 2 + HEADS],
                    in1=ad_ps[:, 0:k * HEADS].rearrange("p (b h) -> p b h", h=HEADS),
                    op=OP.add)
                e2_t = ep.tile([128, MAXH, HEADS], F32, tag="e2")
                nc.vector.tensor_scalar(out=e2_t[:, 0:k, :], in0=e_t[:, 0:k, :],
                                        scalar1=NEG_ATT, scalar2=None, op0=OP.mult)
                nc.vector.tensor_tensor(out=e_t[:, 0:k, :], in0=e_t[:, 0:k, :],
                                        in1=e2_t[:, 0:k, :], op=OP.max)
                nc.scalar.activation(rows[:, 0:k, HC:HC + HEADS],
                                     e_t[:, 0:k, :], ACTF.Exp)
                nc.vector.tensor_tensor(
                    out=rows[:, 0:k, 0:HC].rearrange("p b (h c) -> p b h c", h=HEADS),
                    in0=rows[:, 0:k, 0:HC].rearrange("p b (h c) -> p b h c", h=HEADS),
                    in1=rows[:, 0:k, HC:HC + HEADS].to_broadcast([128, k, HEADS, HID]),
                    op=OP.mult)
                for b in range(k):
                    nc.tensor.matmul(out_ps[:, :], lhsT=mE[:, b, :],
                                     rhs=rows[:, b, 0:HC + HEADS],
                                     start=(b == 0), stop=(b == k - 1))

            with nc.named_scope("L1"):
                ad16_all = cons.tile([128, NT, HEADS], F16)
                nc.scalar.copy(out=ad16_all[:], in_=ad_sh[:])
                bn = bnp.tile([1, 2 * HC], F32, tag="bn")
                accA = cons.tile([128, NT, HC + HEADS], F16)
                for ti, S in enumerate(sched):
                    mT = mp.tile([128, MAXH, 128], FM, tag="mT", bufs=4)
                    mE = mp.tile([128, MAXH, 128], FM, tag="mE", bufs=4)
                    out_ps = ps.tile([128, HC + HEADS], F32, tag="outps", bufs=4)
                    rows = gp.tile([128, MAXH, ROW1], F16, tag="rows", bufs=4)
                    half_tile(S, 0, rows, mT, mE, out_ps, first=ti < 4)
                    nc.vector.tensor_copy(out=accA[:, S["t"], :], in_=out_ps[:, :])
                for ti, S in enumerate(sched):
                    t = S["t"]
                    nt = _tile_nodes(t)
                    mT = mp.tile([128, MAXH, 128], FM, tag="mT", bufs=4)
                    mE = mp.tile([128, MAXH, 128], FM, tag="mE", bufs=4)
                    out_ps = ps.tile([128, HC + HEADS], F32, tag="outps", bufs=4)
                    rows = gp.tile([128, MAXH, ROW1], F16, tag="rows", bufs=4)
                    half_tile(S, 1, rows, mT, mE, out_ps, first=False)
                    tot = gop.tile([128, HC + HEADS], F32, tag="tot")
                    nc.vector.tensor_tensor(out=tot[:], in0=out_ps[:, :],
                                            in1=accA[:, t, :], op=OP.add)
                    dre = ep.tile([128, HEADS], F32, tag="dre")
                    nc.vector.reciprocal(out=dre[:], in_=tot[:, HC:HC + HEADS])
                    gsq = gop.tile([128, 2 * HC], F16, tag="gsq")
                    nc.vector.tensor_tensor(
                        out=gsq[:, 0:HC].rearrange("p (h c) -> p h c", h=HEADS),
                        in0=tot[:, 0:HC].rearrange("p (h c) -> p h c", h=HEADS),
                        in1=dre[:].to_broadcast([128, HEADS, HID]),
                        op=OP.mult)
                    nc.vector.tensor_tensor(out=gsq[0:nt, HC:2 * HC], in0=gsq[0:nt, 0:HC],
                                            in1=gsq[0:nt, 0:HC], op=OP.mult)
                    nc.tensor.matmul(bn[:, :], lhsT=ones16[0:nt, :], rhs=gsq[0:nt, :],
                                     start=(ti == 0), stop=(ti == NT - 1))
                    nc.sync.dma_start(out=g_sh[t * 128:t * 128 + nt, :],
                                        in_=gsq[0:nt, 0:HC])

                # zero the padded tail rows of g_sh
                zr = gop.tile([128, 2 * HC], F16, tag="gsq")
                nc.vector.memset(zr[:], 0.0)
                if NSP > NS:
                    nc.sync.dma_start(out=g_sh[NS:NSP, :], in_=zr[0:NSP - NS, 0:HC])

            # ================= BN stats: AllReduce + affine =================
            with nc.named_scope("BN"):
                bnst = cons.tile([1, 2 * HC], F32)
                nc.vector.tensor_copy(out=bnst[:], in_=bn[:, :])
                nc.sync.dma_start(out=ar_in[:], in_=bnst[:])
                nc.gpsimd.collective_compute(
                    "AllReduce", OP.add, replica_groups=[list(range(R))],
                    ins=[ar_in.opt()], outs=[ar_out.opt()])
                st = cons.tile([1, 2 * HC], F32)
                nc.sync.dma_start(out=st[:], in_=ar_out[:])
                mu = cons.tile([1, HC], F32)
                nc.vector.tensor_scalar(out=mu[:], in0=st[:, 0:HC], scalar1=1.0 / N,
                                        scalar2=None, op0=OP.mult)
                var = cons.tile([1, HC], F32)
                nc.vector.tensor_scalar(out=var[:], in0=st[:, HC:2 * HC], scalar1=1.0 / N,
                                        scalar2=None, op0=OP.mult)
                musq = cons.tile([1, HC], F32)
                nc.vector.tensor_tensor(out=musq[:], in0=mu[:], in1=mu[:], op=OP.mult)
                nc.vector.tensor_tensor(out=var[:], in0=var[:], in1=musq[:], op=OP.subtract)
                nc.vector.tensor_scalar(out=var[:], in0=var[:], scalar1=BN_EPS, scalar2=None,
                                        op0=OP.add)
                rv = cons.tile([1, HC], F32)
                nc.vector.reciprocal(out=rv[:], in_=var[:])
                rs = cons.tile([1, HC], F32)
                nc.scalar.activation(rs[:], rv[:], ACTF.Sqrt)
                gp_ = cons.tile([1, HC], F32)
                nc.vector.tensor_tensor(out=gp_[:], in0=rs[:], in1=gbrow[:, 0:HC], op=OP.mult)
                bp_ = cons.tile([1, HC], F32)
                nc.vector.tensor_tensor(out=bp_[:], in0=gp_[:], in1=mu[:], op=OP.mult)
                nc.vector.tensor_tensor(out=bp_[:], in0=gbrow[:, HC:2 * HC], in1=bp_[:],
                                        op=OP.subtract)
                nc.sync.dma_start(out=gb_d[0:1, 0:HC], in_=gp_[:])
                nc.sync.dma_start(out=gb_d[0:1, HC:2 * HC], in_=bp_[:])

                # read gamma'/beta' transposed: [128,1] per feature half
                gb_cols = gb_d.rearrange("a (f x) -> (a f) x", x=1)
                gpp = [cons.tile([128, 1], F32, name=f"gpp{i}") for i in range(2)]
                bpp = [cons.tile([128, 1], F32, name=f"bpp{i}") for i in range(2)]
                for h in range(2):
                    nc.sync.dma_start(out=gpp[h][:], in_=gb_cols[h * 128:(h + 1) * 128, :])
                    nc.sync.dma_start(out=bpp[h][:], in_=gb_cols[HC + h * 128:HC + (h + 1) * 128, :])

            # ================= P4: y (transposed) and z2 table =================
            with nc.named_scope("P4"):
                chunks = [(i * 512, 512) for i in range(NSP // 512)]
                if NSP % 512:
                    chunks.append((NSP - NSP % 512, NSP % 512))
                for (n0, w) in chunks:
                    yT = []
                    for h in range(2):
                        gT = yp.tile([128, 512], F16, tag=f"gT{h}")
                        nc.sync.dma_start(out=gT[:, 0:w],
                                          in_=g_sh[n0:n0 + w, h * 128:(h + 1) * 128],
                                          transpose=True)
                        nc.vector.tensor_scalar(out=gT[:, 0:w], in0=gT[:, 0:w],
                                                scalar1=gpp[h][:, :], scalar2=bpp[h][:, :],
                                                op0=OP.mult, op1=OP.add)
                        if LRELU:
                            nc.scalar.activation(gT[:, 0:w], gT[:, 0:w],
                                                 ACTF.Lrelu, alpha=NEG_ACT)
                        else:
                            sm = yp.tile([128, 512], F16, tag=f"sm{h}")
                            nc.vector.tensor_scalar(out=sm[:, 0:w], in0=gT[:, 0:w],
                                                    scalar1=NEG_ACT, scalar2=None, op0=OP.mult)
                            nc.vector.tensor_tensor(out=gT[:, 0:w], in0=gT[:, 0:w],
                                                    in1=sm[:, 0:w], op=OP.max)
                        yT.append(gT)
                    for i in range(w // 128):
                        t = (n0 + i * 128) // 128
                        nt = _tile_nodes(t) if t < NT else 0
                        if nt == 0:
                            continue
                        z2 = ps.tile([128, W2C], F32, tag="zps")
                        nc.tensor.matmul(z2[:, :], lhsT=yT[0][:, i * 128:(i + 1) * 128],
                                         rhs=W2e0[:], start=True, stop=False)
                        nc.tensor.matmul(z2[:, :], lhsT=yT[1][:, i * 128:(i + 1) * 128],
                                         rhs=W2e1[:], start=False, stop=True)
                        row2 = rowp.tile([128, ROW2], F16, tag="row2")
                        nc.scalar.copy(out=row2[0:nt, 0:CLASSES], in_=z2[0:nt, 0:CLASSES])
                        nc.scalar.copy(
                            out=row2[:].bitcast(F32)[0:nt, CLASSES // 2:CLASSES // 2 + 1],
                            in_=z2[0:nt, CLASSES:CLASSES + 1])
                        nc.scalar.copy(out=ad2_sh[0:nt, t, :],
                                       in_=z2[0:nt, CLASSES + 1:CLASSES + 2])
                        w2 = ROW2 if SIM_SAFE else CLASSES + 2
                        if SIM_SAFE:
                            nc.vector.memset(row2[0:nt, CLASSES + 2:ROW2], 0.0)
                        nc.sync.dma_start(out=tb2_sh[t * 128:t * 128 + nt, 0:w2],
                                            in_=row2[0:nt, 0:w2])

                nc.gpsimd.collective_compute(
                    "AllGather", OP.bypass, replica_groups=[list(range(R))],
                    ins=[tb2_sh.opt()], outs=[tb2.opt()])

            # ================= L2 edge pass =================
            with nc.named_scope("L2"):
                ad2_16 = cons.tile([128, NT, 1], F16)
                nc.scalar.copy(out=ad2_16[:], in_=ad2_sh[:])
                for ti, S in enumerate(sched):
                    t, b0, nb, kA, kB = S["t"], S["blk0"], S["nb"], S["kA"], S["kB"]
                    nt = _tile_nodes(t)
                    mT = mp.tile([128, MAXK, 128], FM, tag="mT2", bufs=4)
                    nc.sync.dma_start(out=mT[:, 0:nb, :], in_=maskT_d[:, b0:b0 + nb, :])
                    mE = mp.tile([128, MAXK, 128], FM, tag="mE2", bufs=4)
                    nc.sync.dma_start(out=mE[:, 0:nb, :], in_=maskE_d[:, b0:b0 + nb, :])
                    out_ps = ps.tile([128, HC + HEADS], F32, tag="outps", bufs=4)
                    ad_ps = ps1.tile([128, MAXK * HEADS], F32, tag="adps")
                    rows = gp.tile([128, MAXH, ROW1], F16, tag="rows", bufs=4)
                    r2 = rows[:].rearrange("p b (x r) -> p (b x) r", r=ROW2)
                    if kA and S["vA"] < kA * 128:
                        nc.vector.memset(r2[:, kA - 1, :], 0.0)
                    if kB and S["vB"] < kB * 128:
                        nc.vector.memset(r2[:, nb - 1, :], 0.0)
                    for (o, nbc, vc) in gather_calls(kA, S["vA"]):
                        gi = nc.gpsimd.dma_gather(
                            r2[:, o:o + nbc, :], tb2[0:SECT, :],
                            idx_t[:, (b0 + o) * 8:(b0 + o + nbc) * 8],
                            nbc * 128, vc, ROW2, queue_num=nextq())
                        gather_names.append(gi.ins.name)
                    for (o, nbc, vc) in gather_calls(kB, S["vB"]):
                        gi = nc.gpsimd.dma_gather(
                            r2[:, kA + o:kA + o + nbc, :], tb2[SECT:2 * SECT, :],
                            idx_t[:, (b0 + kA + o) * 8:(b0 + kA + o + nbc) * 8],
                            nbc * 128, vc, ROW2, queue_num=nextq())
                        gather_names.append(gi.ins.name)
                    for b in range(nb):
                        nc.tensor.matmul(ad_ps[:, b:b + 1],
                                         lhsT=mT[:, b, :], rhs=ad2_16[:, t, :],
                                         start=True, stop=True)
                    e_t = ep.tile([128, MAXK, HEADS], F32, tag="e")
                    nc.vector.tensor_tensor(
                        out=e_t[:, 0:nb, 0:1],
                        in0=r2.bitcast(F32)[:, 0:nb, CLASSES // 2:CLASSES // 2 + 1],
                        in1=ad_ps[:, 0:nb].rearrange("p (b h) -> p b h", h=1),
                        op=OP.add)
                    if LRELU:
                        nc.scalar.activation(e_t[:, 0:nb, 0:1], e_t[:, 0:nb, 0:1],
                                             ACTF.Lrelu, alpha=NEG_ATT)
                    else:
                        e2_t = ep.tile([128, MAXK, HEADS], F32, tag="e2")
                        nc.vector.tensor_scalar(out=e2_t[:, 0:nb, 0:1], in0=e_t[:, 0:nb, 0:1],
                                                scalar1=NEG_ATT, scalar2=None, op0=OP.mult)
                        nc.vector.tensor_tensor(out=e_t[:, 0:nb, 0:1], in0=e_t[:, 0:nb, 0:1],
                                                in1=e2_t[:, 0:nb, 0:1], op=OP.max)
                    nc.scalar.activation(r2[:, 0:nb, CLASSES:CLASSES + 1],
                                         e_t[:, 0:nb, 0:1], ACTF.Exp)
                    nc.vector.tensor_tensor(
                        out=r2[:, 0:nb, 0:CLASSES],
                        in0=r2[:, 0:nb, 0:CLASSES],
                        in1=r2[:, 0:nb, CLASSES:CLASSES + 1].to_broadcast([128, nb, CLASSES]),
                        op=OP.mult)
                    for b in range(nb):
                        nc.tensor.matmul(out_ps[:, 0:CLASSES + 1], lhsT=mE[:, b, :],
                                         rhs=r2[:, b, 0:CLASSES + 1],
                                         start=(b == 0), stop=(b == nb - 1))
                    dre = ep.tile([128, 1], F32, tag="dre2")
                    nc.vector.reciprocal(out=dre[:], in_=out_ps[:, CLASSES:CLASSES + 1])
                    o32 = gop.tile([128, CLASSES], F32, tag="o32")
                    nc.vector.tensor_scalar(out=o32[:], in0=out_ps[:, 0:CLASSES],
                                            scalar1=dre[:, :], scalar2=None, op0=OP.mult)
                    nc.vector.tensor_tensor(out=o32[:], in0=o32[:], in1=b2rep[:], op=OP.add)
                    nc.sync.dma_start(out=out_d[t * 128:t * 128 + nt, :], in_=o32[0:nt, :])
    nc.compile()
    nc._gather_names = gather_names
    return nc


def lane_qmap(nc):
    """Map build-order gather index -> queue (= scheduled DMASW lane % NQ)."""
    import re
    name_to_build = {n: i for i, n in enumerate(nc._gather_names)}
    qmap = {}
    pos = 0
    for bb in nc.m.functions[0].blocks:
        for inst in bb.instructions:
            if isinstance(inst, mybir.InstDMAGatherAnt):
                si = inst.sync_info() if callable(inst.sync_info) else inst.sync_info
                m = re.findall(r"DMASW(\d+)", str(si))
                lane = int(m[0]) if m else pos % 8
                qmap[name_to_build[inst.name]] = lane % NQ
                pos += 1
    assert len(qmap) == len(nc._gather_names)
    return qmap


def build_two_pass(sched, nblk):
    nc0 = build_program(sched, nblk)
    qmap = lane_qmap(nc0)
    return build_program(sched, nblk, qmap=qmap)


def kernel(x, edge_index, W1, a_src1, a_dst1, b1, gamma, beta, W2, a_src2, a_dst2, b2):
    # b1 cancels inside BatchNorm (constant per-channel shift) -> unused.
    sched, nblk, ins = host_inputs(np.asarray(x), np.asarray(edge_index),
                                   W1, a_src1, a_dst1, gamma, beta,
                                   W2, a_src2, a_dst2, b2)
    nc = build_two_pass(sched, nblk)
    res = bass_utils.run_bass_kernel_spmd(nc, ins, core_ids=list(range(R)))
    out = np.concatenate([res.results[c]["out"] for c in range(R)], axis=0)
    return out.astype(np.float32)
